# revision 11
# baseline (speedup 1.0000x reference)
"""LoRA Linear (x @ W.T + b + 2.0 * (x @ A.T) @ B.T) on 8 TRN2 NeuronCores.

Strategy (fp8 DoubleRow main GEMM + decoupled bf16 LoRA path):
  - Data-parallel: 8192 tokens -> 8 cores x 1024 tokens. W/A/B/b replicated.
  - Main GEMM x @ W.T runs in fp8 e4m3 with perf_mode=DoubleRow (2 k-planes
    per instruction, K=256 per matmul): per [128 token x 512 feature] PSUM
    tile, 32 DR matmuls contract K=4096. fp8 quantization error of the base
    GEMM is ~1.2e-2 of the output norm (the LoRA adjustment dominates the
    output norm), under the 2e-2 gate with margin.
  - LoRA path in bf16 for accuracy: xa = x @ A.T via 256 skinny matmuls with
    A as the *moving* operand (out free dim = 16, so nearly free on the PE),
    accumulated in one PSUM bank as [token, mt, rank]; evicted to SBUF and
    block-transposed by the DVE into [rank, token] layout with a ones row
    (carries the bias).
  - The adjustment (xa @ (2B).T + b) runs as fp8 DoubleRow "ladder"
    matmuls into separate PSUM banks: the 17-row contraction only uses a
    fraction of the 128 partitions, so rows 0..16 carry an exact fp8
    hi/lo split of xa against B8 = e4m3(2B*1024), and rows 32..48 carry
    the same values >>4 against 16*(2B*1024 - B8). One DR instruction per
    256-wide half then equals xa @ 2B*1024 with only eps^2-level error at
    half the bf16 cost. The Activation engine evicts the main PSUM (scaled
    2^-16 to undo the fp8 scales) to a bf16 tile and the DVE adds the adj
    PSUM on top (descaled by 1/1024 in the same instruction). The last 4
    tiles instead accumulate a bf16 adjustment straight into the main PSUM
    (xa is ready by then; the 2^16-scaled B does not fit fp8 range) to
    shorten the pipeline tail.
  - DMA order is hand-scheduled: x8 m-chunks + W o-block halves first so the
    PE starts ~5us in and never starves; x-bf16 chunks stream behind and
    gate only the (cheap, late) xa/adj work.
"""

import numpy as np
import ml_dtypes

import concourse.bass as bass
from concourse import bacc
import concourse.mybir as mybir
import concourse.tile as tile
from concourse.bass_utils import run_bass_kernel_spmd

N_CORES = 8
IN_F = 4096
OUT_F = 4096
RANK = 16
ALPHA = 32.0
SCALING = ALPHA / RANK          # 2.0
B_SZ = 4
S_SZ = 2048
TOK = B_SZ * S_SZ               # 8192
M = TOK // N_CORES              # 1024 tokens per core

P = 128
K2 = IN_F // 256                # 16 double-k tiles
XBC = 8                         # x bf16 chunks (4 k-tiles each)
OB = 8                          # o-blocks of 512
RANK1 = RANK + 1

SX = 32.0                       # fp8 scale for x
SW = 2048.0                     # fp8 scale for W
SXW = SX * SW                   # 65536
SB8 = 1024.0                    # fp8 scale for the adj B ladder

F8 = mybir.dt.float8e4
NP_F8 = ml_dtypes.float8_e4m3   # TRN FP8_EXP4 (max +-240) == ml_dtypes e4m3
BF = mybir.dt.bfloat16
NP_BF = ml_dtypes.bfloat16
F32 = mybir.dt.float32
DR = mybir.MatmulPerfMode.DoubleRow

# schedule tuning (from TimelineSim sweeps)
OW = 512                        # o-block width per PSUM tile
XA_AFTER = 32                   # emit xa block after this many main tiles
ADJ_PER_GROUP = 2               # adj chains drained per subsequent main tile
TAIL_INPSUM = 4                 # last tiles fold adj into the main PSUM
OSB_BUFS = 42                   # bf16 out-tile ring
W8_BUFS = 3                     # W o-block double-buffering depth
PSM_BUFS = 5                    # main PSUM banks

LAST_RESULTS = None             # test.py reads exec_time_ns from here


def _build_nc(ow=None, xa_after=None, adj_per_group=None, tail_inpsum=None,
              osb_bufs=None, w8_bufs=None, psm_bufs=None):
    # All schedule knobs default to the module-level tuned constants.
    ow = ow or OW
    xa_after = xa_after or XA_AFTER
    adj_per_group = adj_per_group or ADJ_PER_GROUP
    tail_inpsum = tail_inpsum or TAIL_INPSUM
    osb_bufs = osb_bufs or OSB_BUFS
    w8_bufs = w8_bufs or W8_BUFS
    psm_bufs = psm_bufs or PSM_BUFS
    nob = OUT_F // ow           # number of o-blocks
    noh = ow // 256             # DR matmuls per k2 per group
    wsp = max(1, ow // 256 // 1)  # w chunk = half block for ow=512, whole else

    nc = bacc.Bacc(None, target_bir_lowering=False)

    abt_d = nc.dram_tensor("abt", [P, 32, RANK], BF, kind="ExternalInput")
    bbt_d = nc.dram_tensor("bbt", [P, 2, OUT_F], F8, kind="ExternalInput")
    bb2_d = nc.dram_tensor("bb2", [RANK1, OUT_F], BF, kind="ExternalInput")
    xbt_d = nc.dram_tensor("xbt", [XBC, P, 4, M], BF, kind="ExternalInput")
    x8t_d = nc.dram_tensor("x8t", [8, P, K2, 2, P], F8, kind="ExternalInput")
    w8t_d = nc.dram_tensor("w8t", [nob, P, K2, 2, ow], F8, kind="ExternalInput")
    out_d = nc.dram_tensor("out", [M, OUT_F], BF, kind="ExternalOutput")

    with tile.TileContext(nc) as tc:
        with (
            tc.tile_pool(name="cst", bufs=1) as cst_pool,
            tc.tile_pool(name="xb", bufs=1) as xb_pool,
            tc.tile_pool(name="x8", bufs=1) as x8_pool,
            tc.tile_pool(name="w8", bufs=w8_bufs) as w8_pool,
            tc.tile_pool(name="xa", bufs=1) as xa_pool,
            tc.tile_pool(name="outs", bufs=osb_bufs) as out_pool,
            tc.tile_pool(name="psmain", bufs=psm_bufs, space="PSUM") as psm_pool,
            tc.tile_pool(name="psadj", bufs=2, space="PSUM") as psa_pool,
            tc.tile_pool(name="psxa", bufs=1, space="PSUM") as psxa_pool,
        ):
            absb = cst_pool.tile([P, 32, RANK], BF, tag="absb")
            bbsb = cst_pool.tile([P, 2, OUT_F], F8, tag="bbsb")
            bbsb2 = cst_pool.tile([RANK1, OUT_F], BF, tag="bbsb2")

            x8sb, xbsb, w8sb = [], [], []
            for c in range(8):
                t = x8_pool.tile([P, K2, 2, P], F8, tag=f"x8{c}")
                x8sb.append(t)
            for c in range(XBC):
                t = xb_pool.tile([P, 4, M], BF, tag=f"xb{c}")
                xbsb.append(t)
            for ob in range(nob):
                t = w8_pool.tile([P, K2, 2, ow], F8, tag="w8")
                w8sb.append(t)

            # DMA emission order == SP issue order == DMA device order.
            KH = K2 // wsp
            wq = [("w", ob, h) for ob in range(nob) for h in range(wsp)]
            dmas = [("x8", 0), wq[0], ("x8", 1)] + wq[1:wsp]
            dmas += [("x8", c) for c in range(2, 8)]
            dmas += wq[wsp:2 * wsp] + [("ab",), ("bb",), ("bb2",)]
            rest_w = wq[2 * wsp:]
            rest_xb = [("xb", c) for c in range(XBC)]
            wi = xi = 0
            while xi < len(rest_xb) or wi < len(rest_w):
                if xi < len(rest_xb):
                    dmas.append(rest_xb[xi]); xi += 1
                if wi < len(rest_w):
                    dmas.append(rest_w[wi]); wi += 1
            for d in dmas:
                if d[0] == "w":
                    _, ob, h = d
                    nc.sync.dma_start(w8sb[ob][:, h * KH:(h + 1) * KH],
                                      w8t_d[ob][:, h * KH:(h + 1) * KH])
                elif d[0] == "x8":
                    nc.sync.dma_start(x8sb[d[1]][:], x8t_d[d[1]])
                elif d[0] == "xb":
                    nc.sync.dma_start(xbsb[d[1]][:], xbt_d[d[1]])
                elif d[0] == "ab":
                    nc.sync.dma_start(absb[:], abt_d[:])
                elif d[0] == "bb":
                    nc.sync.dma_start(bbsb[:], bbt_d[:])
                else:
                    nc.sync.dma_start(bbsb2[:], bb2_d[:])

            # xa_sb[r, m] = (x @ A.T)[m, r] for r<16, row 16 = 1.0 (bias).
            # xam_sb is the pre-transpose [m, mt, r-padded] staging tile;
            # its memset-1.0 padding becomes the ones row after transpose.
            xa_sb = xa_pool.tile([32, M], BF, tag="xasb")
            xam_sb = xa_pool.tile([P, 8, 32], BF, tag="xamsb")
            nc.vector.memset(xam_sb[:], 1.0)
            # fp8 ladder operand for the DoubleRow adj matmul:
            #   rows 0..16  plane0/1 = e4m3(xa) hi / residual lo
            #   rows 32..48 plane0/1 = the same / 16 (exact exponent shift)
            # paired against bbsb rows [B8 ; 16*(B*sb - B8)] so one DR
            # instruction per 256-half computes xa @ (2B*sb) to ~eps^2.
            xa8_sb = xa_pool.tile([P, 2, M], F8, tag="xa8sb")
            nc.vector.memset(xa8_sb[:], 0.0)

            def emit_xa():
                ps = psxa_pool.tile([P, 8, RANK], F32)
                for k in range(32):
                    for mt in range(8):
                        nc.tensor.matmul(
                            ps[:, mt, :],
                            xbsb[k // 4][:, k % 4, mt * P:(mt + 1) * P],
                            absb[:, k, :],
                            start=(k == 0 and mt == 0),
                            stop=(k == 31 and mt == 7),
                            skip_group_check=True)
                nc.scalar.copy(out=xam_sb[:, :, 0:RANK], in_=ps[:])
                for mt in range(8):
                    for bk in range(4):
                        nc.vector.transpose(
                            xa_sb[0:32, mt * P + bk * 32:mt * P + (bk + 1) * 32],
                            xam_sb[bk * 32:(bk + 1) * 32, mt, :])
                nc.scalar.copy(out=xa8_sb[0:RANK1, 0, :], in_=xa_sb[0:RANK1, :])
                nc.vector.scalar_tensor_tensor(
                    xa8_sb[0:RANK1, 1, :], xa_sb[0:RANK1, :], 1.0,
                    xa8_sb[0:RANK1, 0, :],
                    mybir.AluOpType.mult, mybir.AluOpType.subtract)
                nc.scalar.mul(xa8_sb[32:32 + RANK1, 0, :],
                              xa8_sb[0:RANK1, 0, :], 1.0 / 16.0)
                nc.scalar.mul(xa8_sb[32:32 + RANK1, 1, :],
                              xa8_sb[0:RANK1, 1, :], 1.0 / 16.0)

            adj_q = []

            def emit_adj(n):
                for _ in range(n):
                    if not adj_q:
                        return
                    ob, mt, o_sb = adj_q.pop(0)
                    pa = psa_pool.tile([P, ow], F32)
                    for oh in range(ow // 256):
                        nc.tensor.matmul(
                            pa[:, oh * 256:(oh + 1) * 256],
                            xa8_sb[:, :, mt * P:(mt + 1) * P],
                            bbsb[:, :, ob * ow + oh * 256:ob * ow + (oh + 1) * 256],
                            start=(oh == 0), stop=(oh == ow // 256 - 1),
                            perf_mode=DR, skip_group_check=True)
                    nc.vector.scalar_tensor_tensor(
                        o_sb[:], pa[:], 1.0 / SB8, o_sb[:],
                        mybir.AluOpType.mult, mybir.AluOpType.add)
                    nc.sync.dma_start(
                        out_d[mt * P:(mt + 1) * P, ob * ow:(ob + 1) * ow],
                        o_sb[:])

            def emit_dr(ps, ob, mt, k2lo, k2hi, is_start, is_stop):
                for k2 in range(k2lo, k2hi):
                    for oh in range(noh):
                        nc.tensor.matmul(
                            ps[:, oh * 256:(oh + 1) * 256],
                            x8sb[mt][:, k2, :, :],
                            w8sb[ob][:, k2, :, oh * 256:(oh + 1) * 256],
                            start=(is_start and k2 == k2lo and oh == 0),
                            stop=(is_stop and k2 == k2hi - 1
                                  and oh == noh - 1),
                            perf_mode=DR, skip_group_check=True)

            g = 0
            NG = nob * 8
            wave_ps = None
            for ob in range(nob):
                for mt in range(8):
                    in_psum_adj = (NG - (g + 1)) < tail_inpsum
                    if ob == 0 and mt == 0:
                        # Wave-interleave the first two tiles by K halves so
                        # the PE has work while the second W half-block and
                        # x8 chunks stream in.
                        ps = psm_pool.tile([P, ow], F32, tag="psm")
                        wave_ps = psm_pool.tile([P, ow], F32, tag="psm")
                        emit_dr(ps, 0, 0, 0, K2 // 2, True, False)
                        emit_dr(wave_ps, 0, 1, 0, K2 // 2, True, False)
                        emit_dr(ps, 0, 0, K2 // 2, K2, False, True)
                    elif ob == 0 and mt == 1:
                        ps = wave_ps
                        emit_dr(ps, 0, 1, K2 // 2, K2, False, True)
                    else:
                        ps = psm_pool.tile([P, ow], F32, tag="psm")
                        emit_dr(ps, ob, mt, 0, K2, True, not in_psum_adj)
                    o_sb = out_pool.tile([P, ow], BF, tag="osb")
                    if in_psum_adj:
                        nc.tensor.matmul(
                            ps[:], xa_sb[0:RANK1, mt * P:(mt + 1) * P],
                            bbsb2[:, ob * ow:(ob + 1) * ow],
                            start=False, stop=True, skip_group_check=True)
                        nc.scalar.mul(o_sb[:], ps[:], 1.0 / SXW)
                        nc.sync.dma_start(
                            out_d[mt * P:(mt + 1) * P,
                                  ob * ow:(ob + 1) * ow], o_sb[:])
                    else:
                        nc.scalar.mul(o_sb[:], ps[:], 1.0 / SXW)
                        adj_q.append((ob, mt, o_sb))
                    g += 1
                    if g == xa_after:
                        emit_xa()
                    if g > xa_after:
                        emit_adj(adj_per_group)
            emit_adj(len(adj_q))
    nc.compile()
    return nc


_NC_CACHE = None


def _prep_shared(W, b, lora_A, lora_B):
    """Host-side weight prep (replicated across cores)."""
    W8T = np.clip(W.astype(np.float32).T * SW, -240, 240).astype(NP_F8)
    # w8t[ob, p, k2, i, o] = W8T[(k2*2+i)*128+p, ob*OW+o]
    w8t = np.ascontiguousarray(
        W8T.reshape(K2, 2, P, OUT_F // OW, OW).transpose(3, 2, 0, 1, 4))

    # abt[p, k, r] = A[r, k*128+p]
    abt = np.ascontiguousarray(
        lora_A.astype(np.float32).T.reshape(32, P, RANK).transpose(1, 0, 2)
    ).astype(NP_BF)

    bbf = np.empty((RANK1, OUT_F), dtype=np.float32)
    bbf[:RANK] = SCALING * lora_B.astype(np.float32).T
    bbf[RANK] = b.astype(np.float32)
    bb2 = (bbf * SXW).astype(NP_BF)

    # fp8 ladder: rows 0..16 = e4m3(bbf*SB8); rows 32..48 = e4m3(16*resid)
    b8 = np.clip(bbf * SB8, -240, 240).astype(NP_F8)
    bres = np.clip((bbf * SB8 - b8.astype(np.float32)) * 16.0,
                   -240, 240).astype(NP_F8)
    bbt = np.zeros((P, 2, OUT_F), dtype=NP_F8)
    bbt[0:RANK1, 0] = b8
    bbt[0:RANK1, 1] = b8
    bbt[32:32 + RANK1, 0] = bres
    bbt[32:32 + RANK1, 1] = bres
    return w8t, abt, bbt, bb2


def _prep_core(xc):
    """Per-core x prep: xc is [M, IN_F] fp32."""
    xT = np.ascontiguousarray(xc.T)                      # [4096, 1024]
    x8 = np.clip(xT * SX, -240, 240).astype(NP_F8)
    # x8t[mt, p, k2, i, m] = x8[(k2*2+i)*128+p, mt*128+m]
    x8t = np.ascontiguousarray(
        x8.reshape(K2, 2, P, 8, P).transpose(3, 2, 0, 1, 4))
    xb = xT.astype(NP_BF)
    # xbt[c, p, kk, m] = xb[(c*4+kk)*128+p, m]
    xbt = np.ascontiguousarray(
        xb.reshape(XBC, 4, P, M).transpose(0, 2, 1, 3))
    return x8t, xbt


def kernel(x, W, b, lora_A, lora_B, _trace=False):
    global LAST_RESULTS, _NC_CACHE

    w8t, abt, bbt, bb2 = _prep_shared(W, b, lora_A, lora_B)
    x_flat = np.ascontiguousarray(
        np.asarray(x, dtype=np.float32).reshape(TOK, IN_F))

    in_maps = []
    for c in range(N_CORES):
        x8t, xbt = _prep_core(x_flat[c * M:(c + 1) * M])
        in_maps.append({
            "abt": abt, "bbt": bbt, "bb2": bb2, "xbt": xbt,
            "x8t": x8t, "w8t": w8t,
        })

    if _NC_CACHE is None:
        _NC_CACHE = _build_nc()
    nc = _NC_CACHE

    res = run_bass_kernel_spmd(nc, in_maps, core_ids=list(range(N_CORES)),
                               trace=_trace)
    LAST_RESULTS = res

    out = np.concatenate(
        [r["out"].astype(np.float32) for r in res.results], axis=0)
    return out.reshape(B_SZ, S_SZ, OUT_F)
